# revision 1
# baseline (speedup 1.0000x reference)
"""Causal self-attention (B=2, T=2048, C=1024, H=16) on 8 Trainium2 cores.

Sharding: tensor-parallel over heads (2 heads/core). Each core computes
q/k/v for its heads, causal attention, and its slice of the c_proj
contraction; the host sums the 8 partial projection outputs and adds
b_proj.

Device-side layout keeps activations transposed ([feat, tok]) so no
transposes of x/q/k are needed; v is transposed on-chip via DMA-xbar.
Softmax runs over the partition axis of S^T: the denominator comes for
free from a ones-column appended to v in the P@V matmul.
"""

import sys

try:
    import concourse  # noqa: F401
except ImportError:
    sys.path.insert(0, "/opt/trn_rl_repo")

import numpy as np
import ml_dtypes

import concourse.bacc as bacc
import concourse.mybir as mybir
import concourse.tile as tile
from concourse import bass_utils

B, T, C, H, NCORES = 2, 2048, 1024, 16, 8
BT = B * T                  # 4096 tokens total
HPC = H // NCORES           # 2 heads per core
D = C // H                  # 64 head dim
CS = HPC * D                # 128 per-core feature slice
QB = 512                    # q block (free dim per matmul)
KT = 128                    # k tile (partition dim of S^T)
NB = T // QB                # 4 q-blocks per batch
NKT = T // KT               # 16 k-tiles per batch
NCT = C // 128              # 8 contraction tiles over C
BF16 = mybir.dt.bfloat16
F32 = mybir.dt.float32
SCALE = 1.0 / np.sqrt(D)

_built = {}
DEBUG_DUMPS = False
SCHEDULE = "hybrid"   # or "phased"/"wavefront"
PS_Q, PS_S, PS_O = 2, 4, 2
PP_BUFS, OS_BUFS = 6, 6
COPY_ENGINE = "any"  # or "vector"
TRP_MODE = "xbar"   # or "pe"


def _build(repeat=1):
    key = ("nc", repeat)
    if key in _built:
        return _built[key]

    nc = bacc.Bacc("TRN2", target_bir_lowering=False, debug=False,
                   num_devices=NCORES)
    xT = nc.dram_tensor("xT", [C, BT], BF16, kind="ExternalInput")
    wqkv = nc.dram_tensor("wqkv", [C, 3 * CS], BF16, kind="ExternalInput")
    bqkv = nc.dram_tensor("bqkv", [3 * CS, 1], F32, kind="ExternalInput")
    wproj = nc.dram_tensor("wproj", [CS, C], BF16, kind="ExternalInput")
    outT = nc.dram_tensor("outT", [C, BT], F32, kind="ExternalOutput")
    dbg = None
    if DEBUG_DUMPS:
        dbg = {nm: nc.dram_tensor(f"d_{nm}", shp, dt, kind="ExternalOutput")
               for nm, shp, dt in [
                   ("q", [128, BT], BF16), ("k", [128, BT], BF16),
                   ("vT", [128, BT], BF16),
                   ("vn", [128, B * HPC * NKT * 128], BF16),
                   ("y", [128, BT], BF16)]}

    with tile.TileContext(nc) as tc:
        _emit(nc, tc, xT.ap(), wqkv.ap(), bqkv.ap(), wproj.ap(), outT.ap(),
              repeat=repeat, dbg=dbg)
    nc.compile()
    _built[key] = nc
    return nc


def _emit(nc, tc, xT, wqkv, bqkv, wproj, outT, repeat=1, dbg=None):
    from contextlib import ExitStack
    ctx = ExitStack()
    with ctx:
        constp = ctx.enter_context(tc.tile_pool(name="const", bufs=1))
        xp = ctx.enter_context(tc.tile_pool(name="x", bufs=1))
        wp = ctx.enter_context(tc.tile_pool(name="w", bufs=1))
        qkvp = ctx.enter_context(tc.tile_pool(name="qkv", bufs=1))
        vnp = ctx.enter_context(tc.tile_pool(name="vnat", bufs=1))
        ppool = ctx.enter_context(tc.tile_pool(name="pp", bufs=PP_BUFS))
        ypool = ctx.enter_context(tc.tile_pool(name="yt", bufs=1))
        osp = ctx.enter_context(tc.tile_pool(name="ostage", bufs=OS_BUFS))
        rpool = ctx.enter_context(tc.tile_pool(name="rec", bufs=4))
        psQ = ctx.enter_context(tc.tile_pool(name="psQ", bufs=PS_Q, space="PSUM"))
        psS = ctx.enter_context(tc.tile_pool(name="psS", bufs=PS_S, space="PSUM"))
        psO = ctx.enter_context(tc.tile_pool(name="psO", bufs=PS_O, space="PSUM"))

        # ---- constants / weights / inputs ----
        # (weights first: first qkv matmul needs w + one x chunk only)
        w_sb = wp.tile([128, NCT, 3, CS], BF16)
        nc.sync.dma_start(
            w_sb[:],
            wqkv.rearrange("(a p) (m c) -> p a m c", p=128, m=3))
        wp_sb = wp.tile([128, C], BF16)             # W_proj slice [CS=128, C]
        nc.sync.dma_start(wp_sb[:], wproj[:, :])
        bias_sb = wp.tile([128, 3], F32)
        nc.sync.dma_start(bias_sb[:],
                          bqkv.rearrange("(m p) o -> p (m o)", p=128))

        ident = constp.tile([128, 128], BF16)      # for PE-mode transpose
        from concourse.masks import make_identity
        make_identity(nc, ident[:])
        zbias = constp.tile([128, 1], F32)         # explicit exp bias=0:
        nc.gpsimd.memset(zbias[:], 0.0)            # a float bias would pull
        # in a const-AP DMA that queues behind all input DMAs

        # xT c-tiles, loaded per (token-chunk, c-tile) for early start
        x_sb = xp.tile([128, NCT, BT], BF16)
        XC = 512
        for nn_ in range(BT // XC):
            for a in range(NCT):
                nc.sync.dma_start(
                    x_sb[:, a, nn_ * XC:(nn_ + 1) * XC],
                    xT[a * 128:(a + 1) * 128, nn_ * XC:(nn_ + 1) * XC])

        # qkvT activations, [feat 128, tok] each; v produced transposed too
        q_sb = qkvp.tile([128, BT], BF16, tag="q")
        k_sb = qkvp.tile([128, BT], BF16, tag="k")
        vT_sb = qkvp.tile([128, BT], BF16, tag="vT")
        qkv_dst = [q_sb, k_sb, vT_sb]

        # v natural layout per (b, h, ktile): [tok 128, slot 128] with
        # cols [v(64) | ones | pad]: DMA-transpose needs 128-aligned dest
        # offsets, and the ones column makes the P@V matmul also emit the
        # softmax denominator (O' at psum partitions 0:64, denom at 64).
        vn_sb = vnp.tile([128, B, HPC, NKT, 128], BF16)
        nc.gpsimd.memset(vn_sb[:, :, :, :, 64:65], 1.0)

        yT_sb = ypool.tile([128, BT], BF16)         # per-core y^T slice

        def emit_qkv_group(b, n, m):
            tb = b * T
            ps = psQ.tile([128, QB], F32, tag="psQ", name="qkvps")
            for a in range(NCT):
                nc.tensor.matmul(
                    ps[:], w_sb[:, a, m, :],
                    x_sb[:, a, tb + n * QB: tb + (n + 1) * QB],
                    start=(a == 0), stop=(a == NCT - 1))
            nc.any.tensor_scalar_add(
                qkv_dst[m][:, tb + n * QB: tb + (n + 1) * QB],
                ps[:], bias_sb[:, m:m + 1])

        def emit_trp(b, i):
            if TRP_MODE == "pe":
                # PE transpose-mode; one [128,128] covers both heads.
                tb = b * T
                trp = psS.tile([128, KT], BF16, tag="psS", name="trp")
                nc.tensor.transpose(
                    trp[:], vT_sb[:, tb + i * KT: tb + (i + 1) * KT],
                    ident[:])
                for h in range(HPC):
                    eng = nc.vector if COPY_ENGINE == "vector" else nc.any
                    eng.tensor_copy(vn_sb[:, b, h, i, 0:64],
                                    trp[:, h * 64:(h + 1) * 64])
            else:
                # DMA xbar transposes, alone on the scalar-HWDGE queue
                # (mixing with copies on the same queues corrupted).
                tb = b * T
                for h in range(HPC):
                    nc.scalar.dma_start_transpose(
                        vn_sb[:, b, h, i, 0:64],
                        vT_sb[h * 64:(h + 1) * 64,
                              tb + i * KT: tb + (i + 1) * KT])

        def emit_attn_block(b, j):
            tb = b * T
            # O' accumulators: O' at partitions 0:64, denom at 64.
            ops = [psO.tile([128, QB], F32, tag="psO", name=f"op{h}")
                   for h in range(HPC)]
            nkt_j = 4 * (j + 1)
            for i in range(nkt_j):
                c0 = 0 if i < 4 * j else KT * (i - 4 * j)
                w = QB - c0
                for h in range(HPC):
                    hs = h * 64
                    s = psS.tile([128, QB], F32, tag="psS", name="s")
                    nc.tensor.matmul(
                        s[:, 0:w],
                        k_sb[hs:hs + 64, tb + i * KT: tb + (i + 1) * KT],
                        q_sb[hs:hs + 64,
                             tb + j * QB + c0: tb + (j + 1) * QB],
                        start=True, stop=True)
                    p = ppool.tile([128, QB], BF16, tag="pp", name="pp")
                    nc.scalar.activation(
                        p[:, 0:w], s[:, 0:w],
                        mybir.ActivationFunctionType.Exp,
                        bias=zbias[:, 0:1], scale=SCALE)
                    if i >= 4 * j:
                        # zero the strict lower triangle of the 128x128
                        # diagonal block (causal mask) on idle GpSimd;
                        # split PV so its unmasked columns don't wait.
                        nc.gpsimd.affine_select(
                            out=p[:, 0:KT], in_=p[:, 0:KT],
                            compare_op=mybir.AluOpType.is_ge,
                            fill=0.0, base=0, pattern=[[1, KT]],
                            channel_multiplier=-1)
                        if w > KT:
                            nc.tensor.matmul(
                                ops[h][0:65, c0 + KT:QB],
                                vn_sb[:, b, h, i, 0:65], p[:, KT:w],
                                start=(i == 0), stop=False)
                        nc.tensor.matmul(
                            ops[h][0:65, c0:c0 + KT],
                            vn_sb[:, b, h, i, 0:65], p[:, 0:KT],
                            start=False, stop=(i == nkt_j - 1))
                    else:
                        nc.tensor.matmul(
                            ops[h][0:65, c0:QB],
                            vn_sb[:, b, h, i, 0:65], p[:, 0:w],
                            start=(i == 0), stop=(i == nkt_j - 1))

            # normalize: y^T[:, block] = O' / denom. DVE lanes are
            # partition-rigid, so h=1's rows are produced at partitions
            # 0:64 and relocated to 64:128 by GpSimd.
            for h in range(HPC):
                rec = rpool.tile([65, QB], F32, tag="rec", name="rec")
                rec0 = rpool.tile([1, QB], F32, tag="rec0", name="rec0")
                rb = rpool.tile([64, QB], F32, tag="rb", name="rb")
                nc.vector.reciprocal(rec[64:65, :], ops[h][64:65, :])
                # partition_broadcast ucode reads absolute partition 0,
                # so hop the row down first (GpSimd is partition-flexible)
                nc.gpsimd.tensor_copy(rec0[0:1, :], rec[64:65, :])
                nc.gpsimd.partition_broadcast(rb[0:64, :], rec0[0:1, :])
                if h == 0:
                    nc.vector.tensor_mul(
                        yT_sb[0:64, tb + j * QB: tb + (j + 1) * QB],
                        ops[h][0:64, :], rb[0:64, :])
                else:
                    ytmp = rpool.tile([64, QB], BF16, tag="ytmp",
                                      name="ytmp")
                    nc.vector.tensor_mul(
                        ytmp[0:64, :], ops[h][0:64, :], rb[0:64, :])
                    nc.gpsimd.tensor_copy(
                        yT_sb[64:128, tb + j * QB: tb + (j + 1) * QB],
                        ytmp[0:64, :])

        def emit_proj(b, j):
            tb = b * T
            for oc in range(NCT):
                po = psQ.tile([128, QB], F32, tag="psQ", name="po")
                nc.tensor.matmul(
                    po[:], wp_sb[:, oc * 128:(oc + 1) * 128],
                    yT_sb[:, tb + j * QB: tb + (j + 1) * QB],
                    start=True, stop=True)
                ost = osp.tile([128, QB], F32, tag="ostage", name="ost")
                eng = nc.vector if COPY_ENGINE == "vector" else nc.any
                eng.tensor_copy(ost[:], po[:])
                nc.sync.dma_start(
                    outT[oc * 128:(oc + 1) * 128,
                         tb + j * QB: tb + (j + 1) * QB], ost[:])

        # Wavefront emission: attention block j needs only token blocks
        # <= j of q/k/v, so it starts as soon as its slice of qkv/vn is
        # ready; qkv(b=1) and proj are woven in as PE/DVE filler for the
        # ACT-bound attention chain.
        for _rep in range(repeat):
            if SCHEDULE == "hybrid":
                for m in range(3):
                    emit_qkv_group(0, 0, m)
                for i in range(4):
                    emit_trp(0, i)
                emit_attn_block(0, 0)
                for n in range(1, NB):
                    for m in range(3):
                        emit_qkv_group(0, n, m)
                for i in range(4, NKT):
                    emit_trp(0, i)
                for j in range(1, NB):
                    emit_attn_block(0, j)
                    for m in range(3):
                        emit_qkv_group(1, j - 1, m)
                    emit_proj(0, j - 1)
                for i in range(4):
                    emit_trp(1, i)
                emit_attn_block(1, 0)
                for m in range(3):
                    emit_qkv_group(1, NB - 1, m)
                emit_proj(0, NB - 1)
                for j in range(1, NB):
                    for i in range(4 * j, 4 * j + 4):
                        emit_trp(1, i)
                    emit_attn_block(1, j)
                    emit_proj(1, j - 1)
                emit_proj(1, NB - 1)
            elif SCHEDULE == "wavefront":
                for n in range(NB):
                    for m in range(3):
                        emit_qkv_group(0, n, m)
                    for i in range(4 * n, 4 * n + 4):
                        emit_trp(0, i)
                    emit_attn_block(0, n)
                    for m in range(3):
                        emit_qkv_group(1, n, m)
                    emit_proj(0, n)
                for n in range(NB):
                    for i in range(4 * n, 4 * n + 4):
                        emit_trp(1, i)
                    emit_attn_block(1, n)
                    emit_proj(1, n)
            else:
                for n in range(NB):
                    for m in range(3):
                        emit_qkv_group(0, n, m)
                for i in range(NKT):
                    emit_trp(0, i)
                for j in range(NB):
                    emit_attn_block(0, j)
                    for m in range(3):
                        emit_qkv_group(1, j, m)
                    emit_proj(0, j)
                for i in range(NKT):
                    emit_trp(1, i)
                for j in range(NB):
                    emit_attn_block(1, j)
                    emit_proj(1, j)

        if dbg is not None:
            nc.sync.dma_start(dbg["q"].ap(), q_sb[:])
            nc.sync.dma_start(dbg["k"].ap(), k_sb[:])
            nc.sync.dma_start(dbg["vT"].ap(), vT_sb[:])
            nc.sync.dma_start(
                dbg["vn"].ap(),
                vn_sb.rearrange("p a b c d -> p (a b c d)")
                if hasattr(vn_sb, "rearrange") else vn_sb[:])
            nc.sync.dma_start(dbg["y"].ap(), yT_sb[:])


def _host_inputs(x, W_attn, b_attn):
    bf = ml_dtypes.bfloat16
    xTh = np.ascontiguousarray(
        x.reshape(BT, C).T.astype(bf))
    in_maps = []
    for c in range(NCORES):
        lo = c * CS
        wq = W_attn[:, lo:lo + CS]
        wk = W_attn[:, C + lo: C + lo + CS]
        wv = W_attn[:, 2 * C + lo: 2 * C + lo + CS]
        wqkv = np.ascontiguousarray(
            np.concatenate([wq, wk, wv], axis=1).astype(bf))
        bq = np.concatenate([b_attn[lo:lo + CS],
                             b_attn[C + lo: C + lo + CS],
                             b_attn[2 * C + lo: 2 * C + lo + CS]])
        bqkvh = np.ascontiguousarray(
            bq.reshape(3 * CS, 1).astype(np.float32))
        in_maps.append({"xT": xTh, "wqkv": wqkv, "bqkv": bqkvh})
    return in_maps


def kernel(x, W_attn, b_attn, W_proj, b_proj):
    x = np.asarray(x, np.float32)
    W_attn = np.asarray(W_attn, np.float32)
    b_attn = np.asarray(b_attn, np.float32)
    W_proj = np.asarray(W_proj, np.float32)
    b_proj = np.asarray(b_proj, np.float32)

    nc = _build()
    in_maps = _host_inputs(x, W_attn, b_attn)
    bf = ml_dtypes.bfloat16
    for c in range(NCORES):
        in_maps[c]["wproj"] = np.ascontiguousarray(
            W_proj[c * CS:(c + 1) * CS, :].astype(bf))

    res = bass_utils.run_bass_kernel_spmd(
        nc, in_maps, core_ids=list(range(NCORES)))
    acc = np.zeros((C, BT), np.float64)
    for c in range(NCORES):
        acc += res.results[c]["outT"].astype(np.float64)
    out = acc.T.astype(np.float32) + b_proj[None, :]
    return out.reshape(B, T, C)



# revision 9
# speedup vs baseline: 1.0919x; 1.0919x over previous
"""Causal self-attention (B=2, T=2048, C=1024, H=16) on 8 Trainium2 cores.

Sharding: tensor-parallel over heads (2 heads/core). Each core computes
q/k/v for its heads, causal attention, and its slice of the c_proj
contraction; the host sums the 8 partial projection outputs and adds
b_proj.

Device-side layout keeps activations transposed ([feat, tok]) so no
transposes of x/q/k are needed; v is transposed on-chip via DMA-xbar.
Softmax runs over the partition axis of S^T: the denominator comes for
free from a ones-column appended to v in the P@V matmul.

v2 engine budget: ACT runs only exp (+qkv bias evac); DVE does all
PSUM evacuation, the causal mask (mul by a precomputed triangle), and
approx reciprocals; GpSimd only does the tiny normalize hops; both
heads' S matmuls row-pack into one 2-bank PSUM tile so exp covers both
heads in (mostly) one instruction.
"""

import sys

try:
    import concourse  # noqa: F401
except ImportError:
    sys.path.insert(0, "/opt/trn_rl_repo")

import numpy as np
import ml_dtypes

import concourse.bacc as bacc
import concourse.mybir as mybir
import concourse.tile as tile
from concourse import bass_utils

B, T, C, H, NCORES = 2, 2048, 1024, 16, 8
BT = B * T                  # 4096 tokens total
HPC = H // NCORES           # 2 heads per core
D = C // H                  # 64 head dim
CS = HPC * D                # 128 per-core feature slice
QB = 512                    # q block (free dim per matmul)
KT = 128                    # k tile (partition dim of S^T)
NB = T // QB                # 4 q-blocks per batch
NKT = T // KT               # 16 k-tiles per batch
NCT = C // 128              # 8 contraction tiles over C
BF16 = mybir.dt.bfloat16
F32 = mybir.dt.float32
SCALE = 1.0 / np.sqrt(D)

_built = {}


def _build(repeat=1):
    key = ("nc", repeat)
    if key in _built:
        return _built[key]

    nc = bacc.Bacc("TRN2", target_bir_lowering=False, debug=False,
                   num_devices=NCORES)
    xT = nc.dram_tensor("xT", [C, BT], BF16, kind="ExternalInput")
    wqkv = nc.dram_tensor("wqkv", [C, 3 * CS], BF16, kind="ExternalInput")
    bqkv = nc.dram_tensor("bqkv", [3 * CS, 1], F32, kind="ExternalInput")
    wproj = nc.dram_tensor("wproj", [CS, C], BF16, kind="ExternalInput")
    outT = nc.dram_tensor("outT", [C, BT], BF16, kind="ExternalOutput")

    with tile.TileContext(nc) as tc:
        _emit(nc, tc, xT.ap(), wqkv.ap(), bqkv.ap(), wproj.ap(), outT.ap(),
              repeat=repeat)
    nc.compile()
    _built[key] = nc
    return nc


def _emit(nc, tc, xT, wqkv, bqkv, wproj, outT, repeat=1, dbg=None):
    from contextlib import ExitStack
    ctx = ExitStack()
    with ctx:
        constp = ctx.enter_context(tc.tile_pool(name="const", bufs=1))
        xp = ctx.enter_context(tc.tile_pool(name="x", bufs=1))
        wp = ctx.enter_context(tc.tile_pool(name="w", bufs=1))
        qkvp = ctx.enter_context(tc.tile_pool(name="qkv", bufs=1))
        vnp = ctx.enter_context(tc.tile_pool(name="vnat", bufs=1))
        ppool = ctx.enter_context(tc.tile_pool(name="pp", bufs=4))
        ypool = ctx.enter_context(tc.tile_pool(name="yt", bufs=1))
        osp = ctx.enter_context(tc.tile_pool(name="ostage", bufs=4))
        rpool = ctx.enter_context(tc.tile_pool(name="rec", bufs=2))
        psS = ctx.enter_context(tc.tile_pool(name="psS", bufs=2, space="PSUM"))
        psO = ctx.enter_context(tc.tile_pool(name="psO", bufs=2, space="PSUM"))
        psQ = ctx.enter_context(tc.tile_pool(name="psQ", bufs=2, space="PSUM"))

        # ---- constants / weights / inputs ----
        w_sb = wp.tile([128, NCT, 3, CS], BF16)
        nc.sync.dma_start(
            w_sb[:],
            wqkv.rearrange("(a p) (m c) -> p a m c", p=128, m=3))
        wp_sb = wp.tile([128, C], BF16)             # W_proj slice [CS=128, C]
        nc.sync.dma_start(wp_sb[:], wproj[:, :])
        bias_sb = wp.tile([128, 3], F32)
        nc.sync.dma_start(bias_sb[:],
                          bqkv.rearrange("(m p) o -> p (m o)", p=128))

        zbias = constp.tile([128, 1], F32)         # explicit exp bias=0:
        nc.gpsimd.memset(zbias[:], 0.0)            # a float bias would pull
        # in a const-AP DMA that queues behind all input DMAs

        # causal mask for the 128x128 diagonal blocks of S^T: keep k <= q
        mask = constp.tile([128, KT], BF16)
        nc.gpsimd.memset(mask[:], 1.0)
        nc.gpsimd.affine_select(
            out=mask[:], in_=mask[:],
            compare_op=mybir.AluOpType.is_ge,
            fill=0.0, base=0, pattern=[[1, KT]],
            channel_multiplier=-1)

        # xT c-tiles, loaded per (token-chunk, c-tile) for early start
        x_sb = xp.tile([128, NCT, BT], BF16)
        XC = 512
        for nn_ in range(BT // XC):
            for a in range(NCT):
                nc.sync.dma_start(
                    x_sb[:, a, nn_ * XC:(nn_ + 1) * XC],
                    xT[a * 128:(a + 1) * 128, nn_ * XC:(nn_ + 1) * XC])

        # qkvT activations, [feat 128, tok] each; v produced transposed too
        q_sb = qkvp.tile([128, BT], BF16, tag="q")
        k_sb = qkvp.tile([128, BT], BF16, tag="k")
        vT_sb = qkvp.tile([128, BT], BF16, tag="vT")
        qkv_dst = [q_sb, k_sb, vT_sb]

        # v natural layout per (b, h, ktile): [tok 128, slot 128] with
        # cols [v(64) | ones | pad]: DMA-transpose needs 128-aligned dest
        # offsets, and the ones column makes the P@V matmul also emit the
        # softmax denominator (O' at psum partitions 0:64, denom at 64).
        vn_sb = vnp.tile([128, B, HPC, NKT, 128], BF16)
        nc.gpsimd.memset(vn_sb[:, :, :, :, 64:65], 1.0)

        yT_sb = ypool.tile([128, BT], BF16)         # per-core y^T slice

        def emit_qkv_group(b, n, m):
            tb = b * T
            ps = psQ.tile([128, QB], F32, tag="psQ", name="qkvps")
            for a in range(NCT):
                nc.tensor.matmul(
                    ps[:], w_sb[:, a, m, :],
                    x_sb[:, a, tb + n * QB: tb + (n + 1) * QB],
                    start=(a == 0), stop=(a == NCT - 1))
            nc.scalar.add(
                qkv_dst[m][:, tb + n * QB: tb + (n + 1) * QB],
                ps[:], bias_sb[:, m:m + 1])

        def emit_trp(b, i):
            # DMA xbar transposes, alone on the scalar-HWDGE queue
            # (mixing with copies on the same queues corrupted).
            tb = b * T
            for h in range(HPC):
                nc.scalar.dma_start_transpose(
                    vn_sb[:, b, h, i, 0:64],
                    vT_sb[h * 64:(h + 1) * 64,
                          tb + i * KT: tb + (i + 1) * KT])

        def emit_attn_block(b, j):
            tb = b * T
            # O' accumulators: O' at partitions 0:64, denom at 64.
            ops = [psO.tile([65, QB], F32, tag="psO", name=f"op{h}")
                   for h in range(HPC)]
            nkt_j = 4 * (j + 1)
            for i in range(nkt_j):
                c0 = 0 if i < 4 * j else KT * (i - 4 * j)
                w = QB - c0
                # both heads' S^T into one [128, 1024] psum tile
                # (row-packed: h0 on array rows 0:64, h1 on 64:128)
                s = psS.tile([128, 2 * QB], F32, tag="psS", name="s")
                for h in range(HPC):
                    hs = h * 64
                    nc.tensor.matmul(
                        s[:, h * QB + c0: (h + 1) * QB],
                        k_sb[hs:hs + 64, tb + i * KT: tb + (i + 1) * KT],
                        q_sb[hs:hs + 64,
                             tb + j * QB + c0: tb + (j + 1) * QB],
                        start=True, stop=True)
                p = ppool.tile([128, 2 * QB], BF16, tag="pp", name="pp")
                for h in range(HPC):
                    nc.scalar.activation(
                        p[:, h * QB + c0: (h + 1) * QB],
                        s[:, h * QB + c0: (h + 1) * QB],
                        mybir.ActivationFunctionType.Exp,
                        bias=zbias[:, 0:1], scale=SCALE)
                if i >= 4 * j:
                    # zero the strict lower triangle of the 128x128
                    # diagonal block (causal mask) via mul with the
                    # precomputed triangle; split PV so its unmasked
                    # columns don't wait.
                    for h in range(HPC):
                        po = h * QB + c0
                        nc.vector.tensor_mul(
                            p[:, po:po + KT], p[:, po:po + KT], mask[:])
                    for h in range(HPC):
                        po = h * QB
                        if w > KT:
                            nc.tensor.matmul(
                                ops[h][0:65, c0 + KT:QB],
                                vn_sb[:, b, h, i, 0:65],
                                p[:, po + c0 + KT: po + QB],
                                start=(i == 0), stop=False)
                        nc.tensor.matmul(
                            ops[h][0:65, c0:c0 + KT],
                            vn_sb[:, b, h, i, 0:65],
                            p[:, po + c0: po + c0 + KT],
                            start=False, stop=(i == nkt_j - 1))
                else:
                    for h in range(HPC):
                        nc.tensor.matmul(
                            ops[h][0:65, c0:QB],
                            vn_sb[:, b, h, i, 0:65],
                            p[:, h * QB + c0: (h + 1) * QB],
                            start=(i == 0), stop=(i == nkt_j - 1))

            # normalize: y^T[:, block] = O' / denom. DVE lanes are
            # partition-rigid, so h=1's rows are produced at partitions
            # 0:64 and relocated to 64:128 by GpSimd.
            for h in range(HPC):
                rec = rpool.tile([65, QB], F32, tag="rec", name="rec")
                rec0 = rpool.tile([1, QB], F32, tag="rec0", name="rec0")
                rb = rpool.tile([64, QB], F32, tag="rb", name="rb")
                nc.vector.reciprocal(rec[64:65, :], ops[h][64:65, :])
                # partition_broadcast ucode reads absolute partition 0,
                # so hop the row down first (GpSimd is partition-flexible)
                nc.gpsimd.tensor_copy(rec0[0:1, :], rec[64:65, :])
                nc.gpsimd.partition_broadcast(rb[0:64, :], rec0[0:1, :])
                if h == 0:
                    nc.vector.tensor_mul(
                        yT_sb[0:64, tb + j * QB: tb + (j + 1) * QB],
                        ops[h][0:64, :], rb[0:64, :])
                else:
                    ytmp = rpool.tile([64, QB], BF16, tag="ytmp",
                                      name="ytmp")
                    nc.vector.tensor_mul(
                        ytmp[0:64, :], ops[h][0:64, :], rb[0:64, :])
                    nc.gpsimd.tensor_copy(
                        yT_sb[64:128, tb + j * QB: tb + (j + 1) * QB],
                        ytmp[0:64, :])

        def emit_proj(b, j):
            tb = b * T
            for oc2 in range(NCT // 2):
                ost = osp.tile([128, 2 * QB], BF16, tag="ostage", name="ost")
                for k in range(2):
                    oc = oc2 * 2 + k
                    po = psQ.tile([128, QB], F32, tag="psQ", name="po")
                    nc.tensor.matmul(
                        po[:], wp_sb[:, oc * 128:(oc + 1) * 128],
                        yT_sb[:, tb + j * QB: tb + (j + 1) * QB],
                        start=True, stop=True)
                    nc.vector.tensor_copy(ost[:, k * QB:(k + 1) * QB], po[:])
                nc.sync.dma_start(
                    outT[oc2 * 256:(oc2 + 1) * 256,
                         tb + j * QB: tb + (j + 1) * QB]
                    .rearrange("(t p) q -> p t q", t=2),
                    ost[:].rearrange("p (t q) -> p t q", t=2))

        # Wavefront emission: attention block j needs only token blocks
        # <= j of q/k/v, so it starts as soon as its slice of qkv/vn is
        # ready; qkv(b=1) and proj are woven in as PE/DVE filler for the
        # ACT-bound attention chain.
        for _rep in range(repeat):
            for m in range(3):
                emit_qkv_group(0, 0, m)
            for i in range(4):
                emit_trp(0, i)
            emit_attn_block(0, 0)
            for n in range(1, NB):
                for m in range(3):
                    emit_qkv_group(0, n, m)
            for i in range(4, NKT):
                emit_trp(0, i)
            for j in range(1, NB):
                emit_attn_block(0, j)
                for m in range(3):
                    emit_qkv_group(1, j - 1, m)
                emit_proj(0, j - 1)
            for i in range(4):
                emit_trp(1, i)
            emit_attn_block(1, 0)
            for m in range(3):
                emit_qkv_group(1, NB - 1, m)
            emit_proj(0, NB - 1)
            for j in range(1, NB):
                for i in range(4 * j, 4 * j + 4):
                    emit_trp(1, i)
                emit_attn_block(1, j)
                emit_proj(1, j - 1)
            emit_proj(1, NB - 1)

        if dbg is not None:
            nc.sync.dma_start(dbg["q"].ap(), q_sb[:])
            nc.sync.dma_start(dbg["k"].ap(), k_sb[:])
            nc.sync.dma_start(dbg["vT"].ap(), vT_sb[:])
            nc.sync.dma_start(
                dbg["vn"].ap(),
                vn_sb[:].rearrange("p a b c d -> p (a b c d)"))
            nc.sync.dma_start(dbg["y"].ap(), yT_sb[:])
            nc.sync.dma_start(dbg["mask"].ap(), mask[:])


def _host_inputs(x, W_attn, b_attn):
    bf = ml_dtypes.bfloat16
    xTh = np.ascontiguousarray(
        x.reshape(BT, C).T.astype(bf))
    in_maps = []
    for c in range(NCORES):
        lo = c * CS
        wq = W_attn[:, lo:lo + CS]
        wk = W_attn[:, C + lo: C + lo + CS]
        wv = W_attn[:, 2 * C + lo: 2 * C + lo + CS]
        wqkv = np.ascontiguousarray(
            np.concatenate([wq, wk, wv], axis=1).astype(bf))
        bq = np.concatenate([b_attn[lo:lo + CS],
                             b_attn[C + lo: C + lo + CS],
                             b_attn[2 * C + lo: 2 * C + lo + CS]])
        bqkvh = np.ascontiguousarray(
            bq.reshape(3 * CS, 1).astype(np.float32))
        in_maps.append({"xT": xTh, "wqkv": wqkv, "bqkv": bqkvh})
    return in_maps


def kernel(x, W_attn, b_attn, W_proj, b_proj):
    x = np.asarray(x, np.float32)
    W_attn = np.asarray(W_attn, np.float32)
    b_attn = np.asarray(b_attn, np.float32)
    W_proj = np.asarray(W_proj, np.float32)
    b_proj = np.asarray(b_proj, np.float32)

    nc = _build()
    in_maps = _host_inputs(x, W_attn, b_attn)
    bf = ml_dtypes.bfloat16
    for c in range(NCORES):
        in_maps[c]["wproj"] = np.ascontiguousarray(
            W_proj[c * CS:(c + 1) * CS, :].astype(bf))

    res = bass_utils.run_bass_kernel_spmd(
        nc, in_maps, core_ids=list(range(NCORES)))
    acc = np.zeros((C, BT), np.float64)
    for c in range(NCORES):
        acc += res.results[c]["outT"].astype(np.float64)
    out = acc.T.astype(np.float32) + b_proj[None, :]
    return out.reshape(B, T, C)
